# revision 10
# baseline (speedup 1.0000x reference)
"""Bahdanau additive attention (ragged sequence) on 8 Trainium2 NeuronCores.

Reference math (per batch b over sequence l, d=512, a=64):
    parts  = enc @ W_enc + b_attn                        (l, a)
    scores = tanh(parts + hidden @ W_hidden) . v         (l,)
    w      = softmax(scores + mask) over l               (valid: l < lens[b])
    out[b] = sum_l w[l] * enc[l, b, :]                   (512,)

Strategy (batch-parallel over 8 cores, 8 batches each; single pass over enc):
  * Host pre-transposes enc to (b, d, l) as fp16 (halves HBM traffic), so
    stage A (the d-contraction) streams natural [128, <=512] tiles through
    the PE with W_enc chunks stationary.
  * Ragged skipping at 128-column granularity: positions l >= lens[b]
    contribute exactly 0, so whole 128-wide chunks past ceil(lens/128) are
    never loaded or computed. The host sorts batches by chunk count and
    deals them across cores round-robin; the kernel is compiled against the
    per-slot chunk-count template (max across cores per slot).
  * Work is grouped into 1024-col PSUM tiles: one tanh + one exp per group
    (amortizes the ScalarE per-op bubble). The length mask (host-built
    0/-30000 fp8 rows) is accumulated into the score PSUM by an extra PE
    matmul only for 512-col subchunks that reach past the slot's min valid
    length; fully-valid subchunks skip it (the reference's +1 valid shift
    cancels in softmax).
  * exp runs on ACT out of PSUM to BF16 (fp32 range: e^51 is finite, and
    exp(-30000) is exactly 0, so masking stays exact without max
    subtraction); accum_out gives the softmax denominator for free.
  * Stage B: scalar_tensor_tensor is 1x-only on the DVE, so instead the
    product et*p runs as plain tensor_tensor (fp16 x bf16 -> bf16), which
    supports the 2x_1p packed mode (2 elem/cycle/lane), into a per-slot
    scratch [128, dc, n_l]; the sum over l is then one in-place
    tensor_scalar(+accum_out) per (slot, dc), spread across DVE / DVE /
    GpSimd / ACT so no single engine exceeds the DMA roofline.
  * ~8 warmup matmuls on a memset tile at t=0 keep the PE busy through the
    HAM activity window so the clock gate lifts 4/8 -> 8/8 (~1.2 -> 2.4
    GHz) before the real matmuls arrive, instead of ~40us in.
  * Softmax normalization is deferred to a per-column scale at the very
    end; the per-core result [128, BL*DC] is PE-transposed once and DMA'd
    out; the host undoes the batch permutation.
"""
import sys

sys.path.insert(0, "/opt/trn_rl_repo")

from contextlib import ExitStack

import ml_dtypes
import numpy as np

import concourse.bacc as bacc
import concourse.bass as bass  # noqa: F401  (kept for debugging)
import concourse.tile as tile
from concourse import mybir
from concourse.bass_utils import run_bass_kernel_spmd

F32 = mybir.dt.float32
F16 = mybir.dt.float16
BF16 = mybir.dt.bfloat16
F8 = mybir.dt.float8e5

N_CORES = 8
L, B, D, A, H = 2048, 64, 512, 64, 512
BL = B // N_CORES  # local batches per core
DC = D // 128  # d-chunks of 128 partitions
CHUNK = 128  # l-chunk width for ragged skipping
GCOL = 1024  # columns per PSUM score/parts group (2 banks)
SUB = 512  # matmul moving-operand width
NCH = L // CHUNK

# stage-B plan per dc: 'v' = tensor_tensor product + tensor_scalar reduce,
# both on DVE; 'a' = DVE product + ACT Copy-with-accum reduce. (GpSimd's
# Pool engine rejects the TensorScalarPtr family outright.)
STAGE_B = ("v", "v", "v", "a")


def _build_bass(template):
    """template: per-slot (n_chunks128, first_mask_sub512) pairs, len BL."""
    nc = bacc.Bacc(
        "TRN2", target_bir_lowering=False, debug=False, num_devices=N_CORES
    )
    encT = nc.dram_tensor("encT", [BL * D, L], F16, kind="ExternalInput")
    msk = nc.dram_tensor("msk", [1, BL * L], F8, kind="ExternalInput")
    hidT = nc.dram_tensor("hidT", [H, BL], F32, kind="ExternalInput")
    w_enc = nc.dram_tensor("w_enc", [D, A], F16, kind="ExternalInput")
    w_hid = nc.dram_tensor("w_hid", [H, A], F32, kind="ExternalInput")
    b_attn = nc.dram_tensor("b_attn", [A, 1], F32, kind="ExternalInput")
    vrep = nc.dram_tensor("vrep", [A, 128], F16, kind="ExternalInput")
    ones1 = nc.dram_tensor("ones1", [1, 128], F8, kind="ExternalInput")
    ident = nc.dram_tensor("ident", [128, 128], F32, kind="ExternalInput")
    out = nc.dram_tensor("out", [BL, D], F32, kind="ExternalOutput")

    with tile.TileContext(nc) as tc, ExitStack() as ctx:
        const = ctx.enter_context(tc.tile_pool(name="const", bufs=1))
        encp = ctx.enter_context(tc.tile_pool(name="encp", bufs=5))
        tanhp = ctx.enter_context(tc.tile_pool(name="tanhp", bufs=3))
        pp = ctx.enter_context(tc.tile_pool(name="pp", bufs=3))
        scrp = ctx.enter_context(tc.tile_pool(name="scrp", bufs=2))
        smallp = ctx.enter_context(tc.tile_pool(name="smallp", bufs=4))
        resp = ctx.enter_context(tc.tile_pool(name="resp", bufs=1))
        ps_parts = ctx.enter_context(
            tc.tile_pool(name="ps_parts", bufs=2, space="PSUM")
        )
        ps_sc = ctx.enter_context(tc.tile_pool(name="ps_sc", bufs=2, space="PSUM"))

        # ---- one-time constants on the GpSimd-queue HWDGE (keeps both the
        # sync queue free for enc and the ACT queue free for tanh/exp) ----
        def loaded(shape, dtype, dram_ap):
            dst = const.tile(shape, dtype, tag="c_" + dram_ap.tensor.name)
            nc.gpsimd.dma_start(dst[:], dram_ap)
            return dst

        w_enc_sb = loaded(
            [128, DC, A], F16, w_enc.ap().rearrange("(dc p) a -> p dc a", p=128)
        )
        w_hid_sb = loaded(
            [128, DC, A], F32, w_hid.ap().rearrange("(dc p) a -> p dc a", p=128)
        )
        hidT_sb = loaded(
            [128, DC, BL], F32, hidT.ap().rearrange("(dc p) b -> p dc b", p=128)
        )
        vrep_sb = loaded([A, 128], F16, vrep.ap())
        ones1_sb = loaded([1, 128], F8, ones1.ap())
        msk_sb = loaded([1, BL * L], F8, msk.ap())
        ident_sb = loaded([128, 128], F32, ident.ap())
        b_attn_sb = loaded([A, 1], F32, b_attn.ap())

        # ---- PE warmup: ~8 x 512-col matmuls on a memset tile keep the PE
        # busy through the HAM activity window so the clock gate lifts to
        # 8/8 before real work arrives. Results are never read. ----
        warm_sb = const.tile([128, SUB], F16, tag="warm_in")
        nc.vector.memset(warm_sb[:], 0.0)
        for wi in range(8):
            warm_ps = ps_sc.tile([128, GCOL], F32, tag="sc")
            nc.tensor.matmul(
                warm_ps[:, 0:SUB], lhsT=warm_sb[:, 0:128], rhs=warm_sb[:],
                start=True, stop=True,
            )

        # hid = hidden @ W_hidden, transposed to [a, b] (tiny, full fp32)
        hid_t = ps_parts.tile([A, GCOL], F32, tag="parts")
        hid_ps = hid_t[:, 0:BL]
        for dc in range(DC):
            nc.tensor.matmul(
                hid_ps, lhsT=w_hid_sb[:, dc, :], rhs=hidT_sb[:, dc, :],
                start=(dc == 0), stop=(dc == DC - 1),
            )
        hplus_sb = const.tile([A, BL], F32)  # b_attn + hid.T, per-partition bias
        nc.vector.tensor_scalar_add(hplus_sb[:], hid_ps, b_attn_sb[:])

        res = resp.tile([128, BL * DC], F32)  # col j*DC+dc <- accum_out
        s_all = resp.tile([128, BL], F32)  # softmax denominators

        encT_v = encT.ap().rearrange("(b dc p) l -> b dc p l", dc=DC, p=128)

        for j in range(BL):
            C, mask_sub0 = template[j]
            n_l = C * CHUNK
            et = encp.tile([128, DC, L], F16, tag="et")
            for dc in range(DC):
                nc.sync.dma_start(et[:, dc, 0:n_l], encT_v[j, dc, :, 0:n_l])

            p_sb = pp.tile([128, L], BF16, tag="p")
            scr = scrp.tile([128, DC, L], BF16, tag="scr")
            groups = [(g, min(g + GCOL, n_l)) for g in range(0, n_l, GCOL)]
            s_parts = []
            for gi, (c0, c1) in enumerate(groups):
                w = c1 - c0
                # stage A: parts[a, l] accumulated over the 4 d-chunks
                parts_ps = ps_parts.tile([A, GCOL], F32, tag="parts")
                for dc in range(DC):
                    for s0 in range(0, w, SUB):
                        sw = min(SUB, w - s0)
                        nc.tensor.matmul(
                            parts_ps[:, s0 : s0 + sw],
                            lhsT=w_enc_sb[:, dc, :],
                            rhs=et[:, dc, c0 + s0 : c0 + s0 + sw],
                            start=(dc == 0), stop=(dc == DC - 1),
                        )
                th = tanhp.tile([A, GCOL], F16, tag="th")
                nc.scalar.activation(
                    th[:, 0:w], parts_ps[:, 0:w],
                    mybir.ActivationFunctionType.Tanh,
                    bias=hplus_sb[:, j : j + 1],
                )
                # scores broadcast to 128 partitions; length mask only on
                # 512-col subchunks reaching past the slot's min valid len
                sc_ps = ps_sc.tile([128, GCOL], F32, tag="sc")
                for s0 in range(0, w, SUB):
                    sw = min(SUB, w - s0)
                    has_mask = (c0 + s0 + sw) > mask_sub0 * SUB
                    nc.tensor.matmul(
                        sc_ps[:, s0 : s0 + sw], lhsT=vrep_sb[:],
                        rhs=th[:, s0 : s0 + sw],
                        start=True, stop=not has_mask,
                    )
                    if has_mask:
                        nc.tensor.matmul(
                            sc_ps[:, s0 : s0 + sw], lhsT=ones1_sb[:],
                            rhs=msk_sb[:, j * L + c0 + s0 : j * L + c0 + s0 + sw],
                            start=False, stop=True,
                        )
                sh = smallp.tile([128, 1], F32, tag=f"sh{gi}")
                nc.scalar.activation(
                    p_sb[:, c0:c1], sc_ps[:, 0:w],
                    mybir.ActivationFunctionType.Exp, accum_out=sh[:],
                )
                s_parts.append(sh)
                # stage B product for this group: 2x_1p tensor_tensor on
                # DVE; 'g' dcs instead run fused on GpSimd per-slot below
                for dc in range(DC):
                    if STAGE_B[dc] != "g":
                        nc.vector.tensor_mul(
                            scr[:, dc, c0:c1], et[:, dc, c0:c1], p_sb[:, c0:c1]
                        )

            # softmax denominator for this slot
            if len(s_parts) == 2:
                nc.gpsimd.tensor_add(s_all[:, j : j + 1], s_parts[0][:], s_parts[1][:])
            else:
                nc.gpsimd.tensor_copy(s_all[:, j : j + 1], s_parts[0][:])

            # stage B reduction per dc (see STAGE_B)
            for dc in range(DC):
                acc = res[:, j * DC + dc : j * DC + dc + 1]
                eng = STAGE_B[dc]
                if eng == "g":
                    nc.gpsimd.scalar_tensor_tensor(
                        out=scr[:, dc, 0:n_l], in0=et[:, dc, 0:n_l],
                        scalar=1.0, in1=p_sb[:, 0:n_l],
                        op0=mybir.AluOpType.mult, op1=mybir.AluOpType.mult,
                        accum_out=acc,
                    )
                elif eng == "a":
                    nc.scalar.activation(
                        scr[:, dc, 0:n_l], scr[:, dc, 0:n_l],
                        mybir.ActivationFunctionType.Copy, accum_out=acc,
                    )
                else:
                    nc.vector.tensor_scalar(
                        scr[:, dc, 0:n_l], scr[:, dc, 0:n_l], 1.0, None,
                        op0=mybir.AluOpType.mult, op1=mybir.AluOpType.add,
                        accum_out=acc,
                    )

        # normalize by 1/S_j, then transpose + write out
        r_all = resp.tile([128, BL], F32)
        nc.vector.reciprocal(r_all[:], s_all[:])
        for j in range(BL):
            nc.vector.tensor_scalar_mul(
                res[:, j * DC : (j + 1) * DC],
                res[:, j * DC : (j + 1) * DC],
                r_all[:, j : j + 1],
            )
        t_t = ps_parts.tile([A, GCOL], F32, tag="parts")
        t_ps = t_t[0 : BL * DC, 0:128]
        nc.tensor.transpose(t_ps, res[:], ident_sb[:])
        out_sb = resp.tile([BL * DC, 128], F32)
        nc.vector.tensor_copy(out_sb[:], t_ps)
        nc.sync.dma_start(out.ap().rearrange("b (dc x) -> (b dc) x", x=128), out_sb[:])

    nc.compile()
    return nc


_NC_CACHE = {}


def _get_nc(template):
    key = tuple((int(c), int(m)) for c, m in template)
    if key not in _NC_CACHE:
        _NC_CACHE[key] = _build_bass(key)
    return _NC_CACHE[key]


def _plan(lens):
    """Balance batches across cores by valid-chunk count.

    Returns (assign, template): assign[c][j] = original batch index handled
    by core c, slot j; template[j] = (chunks, first_mask_sub) compiled for
    slot j (chunks = max need over cores; first_mask_sub = first 512-col
    subchunk index containing any invalid position for any core's batch).
    """
    lens = np.maximum(np.asarray(lens), 1)
    chunks = np.minimum(np.ceil(lens / CHUNK).astype(int), NCH)
    order = np.argsort(-chunks, kind="stable")  # descending need
    # rank r (0=biggest group) -> slot position: put the second-smallest
    # group first so the pipeline primes fast, keep the smallest last so the
    # drain tail is short, biggest groups in the middle.
    ranks = list(range(BL))
    slot_ranks = [ranks[-2]] + ranks[: BL - 2] + [ranks[-1]]
    assign = [
        [int(order[r * N_CORES + c]) for r in slot_ranks] for c in range(N_CORES)
    ]
    template = []
    for si, r in enumerate(slot_ranks):
        group = [int(order[r * N_CORES + c]) for c in range(N_CORES)]
        cmax = int(chunks[order[r * N_CORES]])
        min_len = int(min(lens[b] for b in group))
        template.append((cmax, min_len // SUB))
    return assign, tuple(template)


def prepare_in_maps(enc_outputs, lens, hidden_states, W_enc, b_attn, W_hidden, v):
    """Host-side sharding + layout transforms. Returns (in_maps, assign)."""
    enc_outputs = np.asarray(enc_outputs, dtype=np.float32)
    lens = np.asarray(lens, dtype=np.int32)
    hidden_states = np.asarray(hidden_states, dtype=np.float32)
    W_enc = np.asarray(W_enc, dtype=np.float32)
    b_attn = np.asarray(b_attn, dtype=np.float32)
    W_hidden = np.asarray(W_hidden, dtype=np.float32)
    v = np.asarray(v, dtype=np.float32)

    assign, template = _plan(lens)

    # (L, B, D) -> (B, D, L), contiguous, fp16 (halves the HBM traffic; the
    # softmax weights and stage-B accumulation stay bf16/fp32)
    encT = np.ascontiguousarray(enc_outputs.transpose(1, 2, 0).astype(np.float16))
    w_enc_r = W_enc.astype(np.float16)
    vrep = np.ascontiguousarray(np.repeat(v.astype(np.float16)[:, None], 128, axis=1))
    ones1 = np.ones((1, 128), dtype=ml_dtypes.float8_e5m2)
    ident = np.eye(128, dtype=np.float32)
    b_attn_c = np.ascontiguousarray(b_attn[:, None])

    # length mask rows: 0 where l < lens[b], -30000 where l >= lens[b]
    li = np.arange(L, dtype=np.int32)[None, :]
    mask_full = np.where(li < lens[:, None], 0.0, -30000.0).astype(
        ml_dtypes.float8_e5m2
    )  # (B, L)

    hiddenT = hidden_states.T  # (H, B)

    in_maps = []
    for c in range(N_CORES):
        bs = assign[c]
        in_maps.append(
            {
                "encT": np.ascontiguousarray(encT[bs]).reshape(BL * D, L),
                "msk": np.ascontiguousarray(mask_full[bs]).reshape(1, BL * L),
                "hidT": np.ascontiguousarray(hiddenT[:, bs]),
                "w_enc": w_enc_r,
                "w_hid": W_hidden,
                "b_attn": b_attn_c,
                "vrep": vrep,
                "ones1": ones1,
                "ident": ident,
            }
        )
    return in_maps, assign, template


def _run(inputs_np, trace=False):
    in_maps, assign, template = prepare_in_maps(**inputs_np)
    nc = _get_nc(template)
    res = run_bass_kernel_spmd(
        nc, in_maps, core_ids=list(range(N_CORES)), trace=trace
    )
    out = np.empty((B, D), dtype=np.float32)
    for c in range(N_CORES):
        rows = res.results[c]["out"]
        for j in range(BL):
            out[assign[c][j]] = rows[j]
    return out, res


def kernel(enc_outputs, lens, hidden_states, W_enc, b_attn, W_hidden, v, **kwargs):
    out, _ = _run(
        dict(
            enc_outputs=enc_outputs, lens=lens, hidden_states=hidden_states,
            W_enc=W_enc, b_attn=b_attn, W_hidden=W_hidden, v=v,
        )
    )
    return out


def kernel_traced(enc_outputs, lens, hidden_states, W_enc, b_attn, W_hidden, v):
    """Like kernel() but returns (output, BassKernelResults with trace)."""
    return _run(
        dict(
            enc_outputs=enc_outputs, lens=lens, hidden_states=hidden_states,
            W_enc=W_enc, b_attn=b_attn, W_hidden=W_hidden, v=v,
        ),
        trace=True,
    )


# revision 38
# speedup vs baseline: 1.7638x; 1.7638x over previous
"""Bahdanau additive attention (ragged sequence) on 8 Trainium2 NeuronCores.

Reference math (per batch b over sequence l, d=512, a=64):
    parts  = enc @ W_enc + b_attn                        (l, a)
    scores = tanh(parts + hidden @ W_hidden) . v         (l,)
    w      = softmax(scores + mask) over l               (valid: l < lens[b])
    out[b] = sum_l w[l] * enc[l, b, :]                   (512,)

Strategy (batch-parallel over 8 cores, 8 batches each; single pass over enc):
  * Host pre-transposes enc to (b, d, l) as fp16 (halves HBM traffic), so
    stage A (the d-contraction) streams natural [128, <=512] tiles through
    the PE with W_enc chunks stationary.
  * Ragged skipping at 128-column granularity: positions l >= lens[b]
    contribute exactly 0, so whole 128-wide chunks past ceil(lens/128) are
    never loaded or computed. The host sorts batches by chunk count and
    deals them across cores round-robin; the kernel is compiled against the
    per-slot chunk-count template (max across cores per slot).
  * Work is grouped into 1024-col PSUM tiles: one tanh + one exp per group
    (amortizes the ScalarE per-op bubble). The length mask (host-built
    0/-30000 fp8 rows) is accumulated into the score PSUM by an extra PE
    matmul only for 512-col subchunks that reach past the slot's min valid
    length; fully-valid subchunks skip it (the reference's +1 valid shift
    cancels in softmax).
  * exp runs on ACT out of PSUM to BF16 (fp32 range: e^51 is finite, and
    exp(-30000) is exactly 0, so masking stays exact without max
    subtraction); accum_out gives the softmax denominator for free.
  * Stage B: scalar_tensor_tensor is 1x-only on the DVE, so instead the
    product et*p runs as plain tensor_tensor (fp16 x bf16 -> bf16), which
    supports the 2x_1p packed mode (2 elem/cycle/lane), into a per-slot
    scratch [128, dc, n_l]; the sum over l is then one in-place
    tensor_scalar(+accum_out) per (slot, dc), spread across DVE / DVE /
    GpSimd / ACT so no single engine exceeds the DMA roofline.
  * ~8 warmup matmuls on a memset tile at t=0 keep the PE busy through the
    HAM activity window so the clock gate lifts 4/8 -> 8/8 (~1.2 -> 2.4
    GHz) before the real matmuls arrive, instead of ~40us in.
  * Softmax normalization is deferred to a per-column scale at the very
    end; the per-core result [128, BL*DC] is PE-transposed once and DMA'd
    out; the host undoes the batch permutation.
"""
import sys

sys.path.insert(0, "/opt/trn_rl_repo")

from contextlib import ExitStack

import ml_dtypes
import numpy as np

import concourse.bacc as bacc
import concourse.bass as bass  # noqa: F401  (kept for debugging)
import concourse.bass_isa as bass_isa
import concourse.dve_ops as dve_ops
import concourse.tile as tile
from concourse import mybir
from concourse.bass_utils import run_bass_kernel_spmd
from concourse.dve_spec import Spec, Src0, Src1, Zero, lower as dve_lower
from concourse.dve_uop import (
    AluInp,
    AluOp,
    DelayInp,
    DveOpSpec,
    InpSel,
    OutPath,
    OutSel,
    Trigger,
    UopConfig,
)
from operator import add as _op_add

F32 = mybir.dt.float32
F16 = mybir.dt.float16
BF16 = mybir.dt.bfloat16
F8 = mybir.dt.float8e5

N_CORES = 8
L, B, D, A, H = 2048, 64, 512, 64, 512
BL = B // N_CORES  # local batches per core
DC = D // 128  # d-chunks of 128 partitions
CHUNK = 128  # l-chunk width for ragged skipping
GCOL = 1024  # columns per PSUM score/parts group (2 banks)
SUB = 512  # matmul moving-operand width
NCH = L // CHUNK
PADC = 16  # zero-padded tail columns for the MUL_ACC_2X ripple readback

# ---------------------------------------------------------------------------
# MUL_ACC_2X: custom DVE op computing accum_out[p] = sum_k in0[p,k]*in1[p,k]
# in one pass. The stock fused op (scalar_tensor_tensor) has no fast-mode uop
# programs, so it runs at 1 elem/cycle/lane; this op adds a hand-written
# 2X_1PORT variant (the design doc's T1 mechanism): each cycle reads one
# packed 32-bit word per port (two 16-bit elements), multiplies lo and hi
# pairs on two ALU stages, pair-sums them, and feeds one running-total
# accumulator (block-3 CURR_ALU_OUT feedback, a-flop chain to block 7, read
# back by the auto-appended DVE_READ_ACCUMULATOR2). Products are formed in
# fp32 (fp16 x bf16 is exact) and never round-trip through the bf16 `out`
# tensor - the 2x program writes no outputs at all.


def _mul_acc_2x_uops():
    lanes = [InpSel.SRC_0, InpSel.SRC_1, InpSel.SRC_0_HI, InpSel.SRC_1_HI,
             InpSel.ZERO]

    def base():
        u = UopConfig()
        for i, s in enumerate(lanes):
            u.enable_input(s, i)
        u.accum_enabled = 1
        # body: lo/hi products; lo rides delay0, hi rides delay1 to block 7
        u.datapath_config[0].enable_alu(
            AluOp.MULTIPLY, AluInp.PREV_ALU_OUT, AluInp.PREV_DELAY_0
        ).pass_through_delay(1, 2, 3)
        u.datapath_config[1].enable_alu(
            AluOp.MULTIPLY, AluInp.PREV_DELAY_1, AluInp.PREV_DELAY_2
        ).enable_delay_from_src(DelayInp.PREV_ALU_OUT, 0).pass_through_delay(3)
        u.datapath_config[2].enable_delay_from_src(
            DelayInp.PREV_ALU_OUT, 1
        ).pass_through_delay(0, 3)
        for b in range(3, 8):
            u.datapath_config[b].pass_through_delay(0, 1)
        for b in range(4, 8):
            u.datapath_config[b].pass_through_alu()
            u.datapath_config[b].alu_out_a_enable = 1
        return u

    seed = base()
    seed.repeat_count = 1
    seed.trigger = (Trigger.COUNT, Trigger.NONE, Trigger.NONE)
    seed.next_uop = (1, 0, 0)
    seed.datapath_config[2].pass_through_alu()
    seed.datapath_config[3].enable_alu(
        AluOp.BYPASS, AluInp.PREV_DELAY_3, AluInp.PREV_DELAY_3
    )
    seed.datapath_config[3].alu_out_a_enable = 1

    steady = base()
    steady.trigger = (Trigger.SRC_TENSOR_DONE, Trigger.NONE, Trigger.NONE)
    steady.require_inp0 = 1
    steady.require_inp1 = 1
    steady.datapath_config[2].enable_alu(
        AluOp.ADD, AluInp.PREV_ALU_OUT, AluInp.PREV_DELAY_0
    )
    steady.datapath_config[3].enable_alu(
        AluOp.ADD, AluInp.CURR_ALU_OUT, AluInp.PREV_ALU_OUT
    )
    steady.datapath_config[3].alu_out_a_enable = 1
    # Block 4 (BYPASS of the acc ripple) re-latches the running total into
    # its swap flop every cycle; swap flops persist across instructions
    # (even through DRAIN), so ACC_READ can recover the final total. The
    # >=10 zero-padded tail elements let the last real pair-sums ripple
    # into it before the stream ends.
    steady.datapath_config[4].swap_enable = 1
    steady.enable_output(OutSel.ALU_OUT, OutPath.WR0_LO)
    steady.enable_output(OutSel.DELAY_1, OutPath.WR0_HI)
    for u in (seed, steady):
        u.validate("v3")
    return [seed, steady]





def _mul_acc_ref(in0, in1, s0, s1, imm2):
    b = (in0.astype(np.float32) * in1.astype(np.float32)).astype(np.float32)
    return b, b.reshape(b.shape[0], -1).sum(axis=-1, keepdims=True)


class _HandUopOp(dve_ops.DveOp):
    """DveOp whose uop programs are hand-written (no sha pin)."""

    def compile(self, ver):
        key = (self.name, ver)
        if (r := dve_ops._COMPILE_CACHE.get(key)) is not None:
            return r
        assert ver == "v3", f"{self.name} authored for TRN2 (v3) only"
        uops = dve_lower(self.spec, ver=ver)
        # the lowered 1x fallback accumulates at block 1 and bypasses it down
        # the ALU chain; emit the ripple (not the per-element body) so the
        # pad-region readback works for either program
        uops[1].out[OutPath.WR0_LO] = OutSel.ALU_OUT
        result = DveOpSpec(
            name=self.name,
            opcode=dve_ops.get_dve_sub_opcode(self.name),
            uops=uops,
            rd1_en=True,
            uops_2x=_mul_acc_2x_uops(),
        )
        dve_ops._COMPILE_CACHE[key] = result
        return result


def _register(name, spec):
    if name in dve_ops._SUB_OPCODE_FOR_NAME:
        return next(o for o in dve_ops.OPS if o.name == name)
    op = _HandUopOp(name=name, spec=spec, subdim=False, uops_sha={})
    dve_ops.OPS.append(op)
    row = dve_ops._CUSTOM_DVE_ROW_BASE + len(dve_ops.OPS) - 1
    assert row < 0x20
    dve_ops._SUB_OPCODE_FOR_NAME[op.name] = row
    dve_ops.CUSTOM_DVE_SPECS[op.name] = op.spec
    return op


MUL_ACC = _register(
    "MUL_ACC_2X",
    Spec(body=Src0 * Src1, accum=_op_add, accum_init=Zero, reference=_mul_acc_ref),
)


MUL_ACC_PERF_MAX = 1  # 0 = force REGULAR program (debug)


def _emit_custom(nc, op, perf_max, ins_aps, out_ap, rd1_en):
    v = nc.vector
    if op.name not in nc.m.ant_custom_dve_ops:
        nc.m.ant_custom_dve_ops = sorted({*nc.m.ant_custom_dve_ops, op.name})
    shape = bass_isa.CustomDveShape.TTSS
    isa_opcode = nc.isa.Opcode[
        f"NEURON_ISA_TPB_OPCODE_CUSTOM_DVE_ANT_{shape.slot()}"
    ].value
    zero = mybir.ImmediateValue(dtype=mybir.dt.float32, value=0.0)
    return v.add_instruction(
        bass_isa.InstCustomDveAnt(
            name=nc.get_next_instruction_name(),
            op_name=op.name,
            rd1_en=rd1_en,
            subdim=0,
            imm2=0.0,
            shape=shape,
            row=dve_ops.get_dve_sub_opcode(op.name),
            isa_opcode=isa_opcode,
            perf_max=perf_max,
            ins=[v.lower_ap(a, for_isa=True, opt=True) for a in ins_aps]
            + [zero, zero],
            outs=[v.lower_ap(out_ap, for_isa=True, opt=True)],
        )
    )


def emit_mul_acc(nc, out_ap, in0_ap, in1_ap):
    """out_ap[p, 2i] (2x) / out_ap[p, i] (1x fallback) = running total of
    sum_k in0[p, k] * in1[p, k], at 2 elem/cycle/lane in the 2x program.

    The operand tails must be zero (in1) and finite (in0) for >=12 elements
    so the final total is stable in the pad region: read it at column
    n-4 of a 16-padded stream (even, >= 8 past the last real element)."""
    return _emit_custom(
        nc, MUL_ACC, MUL_ACC_PERF_MAX, [in0_ap, in1_ap], out_ap, True
    )


def _build_bass(template):
    """template: per-slot (n_chunks128, first_mask_sub512) pairs, len BL."""
    nc = bacc.Bacc(
        "TRN2", target_bir_lowering=False, debug=False, num_devices=N_CORES
    )
    encT = nc.dram_tensor("encT", [BL * D, L], F16, kind="ExternalInput")
    msk = nc.dram_tensor("msk", [1, BL * L], F8, kind="ExternalInput")
    hidT = nc.dram_tensor("hidT", [H, BL], F32, kind="ExternalInput")
    w_enc = nc.dram_tensor("w_enc", [D, A], F16, kind="ExternalInput")
    w_hid = nc.dram_tensor("w_hid", [H, A], F32, kind="ExternalInput")
    b_attn = nc.dram_tensor("b_attn", [A, 1], F32, kind="ExternalInput")
    vrep = nc.dram_tensor("vrep", [A, 128], F16, kind="ExternalInput")
    ones1 = nc.dram_tensor("ones1", [1, 128], F8, kind="ExternalInput")
    ident = nc.dram_tensor("ident", [128, 128], F32, kind="ExternalInput")
    out = nc.dram_tensor("out", [BL, D], F32, kind="ExternalOutput")

    with tile.TileContext(nc) as tc, ExitStack() as ctx:
        const = ctx.enter_context(tc.tile_pool(name="const", bufs=1))
        encp = ctx.enter_context(tc.tile_pool(name="encp", bufs=5))
        tanhp = ctx.enter_context(tc.tile_pool(name="tanhp", bufs=3))
        pp = ctx.enter_context(tc.tile_pool(name="pp", bufs=3))
        scrp = ctx.enter_context(tc.tile_pool(name="scrp", bufs=2))
        smallp = ctx.enter_context(tc.tile_pool(name="smallp", bufs=4))
        resp = ctx.enter_context(tc.tile_pool(name="resp", bufs=1))
        ps_parts = ctx.enter_context(
            tc.tile_pool(name="ps_parts", bufs=2, space="PSUM")
        )
        ps_sc = ctx.enter_context(tc.tile_pool(name="ps_sc", bufs=2, space="PSUM"))

        # ---- one-time constants on the GpSimd-queue HWDGE (keeps both the
        # sync queue free for enc and the ACT queue free for tanh/exp) ----
        def loaded(shape, dtype, dram_ap):
            dst = const.tile(shape, dtype, tag="c_" + dram_ap.tensor.name)
            nc.gpsimd.dma_start(dst[:], dram_ap)
            return dst

        w_enc_sb = loaded(
            [128, DC, A], F16, w_enc.ap().rearrange("(dc p) a -> p dc a", p=128)
        )
        w_hid_sb = loaded(
            [128, DC, A], F32, w_hid.ap().rearrange("(dc p) a -> p dc a", p=128)
        )
        hidT_sb = loaded(
            [128, DC, BL], F32, hidT.ap().rearrange("(dc p) b -> p dc b", p=128)
        )
        vrep_sb = loaded([A, 128], F16, vrep.ap())
        ones1_sb = loaded([1, 128], F8, ones1.ap())
        msk_sb = loaded([1, BL * L], F8, msk.ap())
        ident_sb = loaded([128, 128], F32, ident.ap())
        b_attn_sb = loaded([A, 1], F32, b_attn.ap())

        # ---- PE warmup: ~8 x 512-col matmuls on a memset tile keep the PE
        # busy through the HAM activity window so the clock gate lifts to
        # 8/8 before real work arrives. Results are never read. ----
        warm_sb = const.tile([128, SUB], F16, tag="warm_in")
        nc.vector.memset(warm_sb[:], 0.0)
        for wi in range(16):
            warm_ps = ps_sc.tile([128, GCOL], F32, tag="sc")
            nc.tensor.matmul(
                warm_ps[:, 0:SUB], lhsT=warm_sb[:, 0:128], rhs=warm_sb[:],
                start=True, stop=True,
            )

        # hid = hidden @ W_hidden, transposed to [a, b] (tiny, full fp32)
        hid_t = ps_parts.tile([A, GCOL], F32, tag="parts")
        hid_ps = hid_t[:, 0:BL]
        for dc in range(DC):
            nc.tensor.matmul(
                hid_ps, lhsT=w_hid_sb[:, dc, :], rhs=hidT_sb[:, dc, :],
                start=(dc == 0), stop=(dc == DC - 1),
            )
        hplus_sb = const.tile([A, BL], F32)  # b_attn + hid.T, per-partition bias
        nc.vector.tensor_scalar_add(hplus_sb[:], hid_ps, b_attn_sb[:])

        res = resp.tile([128, BL * DC], F32)  # col j*DC+dc <- accum_out
        s_all = resp.tile([128, BL], F32)  # softmax denominators

        encT_v = encT.ap().rearrange("(b dc p) l -> b dc p l", dc=DC, p=128)

        for j in range(BL):
            C, mask_sub0 = template[j]
            n_l = C * CHUNK
            et = encp.tile([128, DC, L + PADC], F16, tag="et")
            for dc in range(DC):
                nc.sync.dma_start(et[:, dc, 0:n_l], encT_v[j, dc, :, 0:n_l])

            p_sb = pp.tile([128, L + PADC], BF16, tag="p")
            scr = scrp.tile([128, DC, L + PADC], BF16, tag="scr")
            # zero tails: products there contribute 0 and give the ripple
            # time to settle (GpSimd memset; eff 1.0, keeps DVE clean)
            nc.gpsimd.memset(et[:, :, n_l : n_l + PADC], 0.0)
            nc.gpsimd.memset(p_sb[:, n_l : n_l + PADC], 0.0)
            groups = [(g, min(g + GCOL, n_l)) for g in range(0, n_l, GCOL)]
            s_parts = []
            for gi, (c0, c1) in enumerate(groups):
                w = c1 - c0
                # stage A: parts[a, l] accumulated over the 4 d-chunks
                parts_ps = ps_parts.tile([A, GCOL], F32, tag="parts")
                for dc in range(DC):
                    for s0 in range(0, w, SUB):
                        sw = min(SUB, w - s0)
                        nc.tensor.matmul(
                            parts_ps[:, s0 : s0 + sw],
                            lhsT=w_enc_sb[:, dc, :],
                            rhs=et[:, dc, c0 + s0 : c0 + s0 + sw],
                            start=(dc == 0), stop=(dc == DC - 1),
                        )
                th = tanhp.tile([A, GCOL], F16, tag="th")
                nc.scalar.activation(
                    th[:, 0:w], parts_ps[:, 0:w],
                    mybir.ActivationFunctionType.Tanh,
                    bias=hplus_sb[:, j : j + 1],
                )
                # scores broadcast to 128 partitions; length mask only on
                # 512-col subchunks reaching past the slot's min valid len
                sc_ps = ps_sc.tile([128, GCOL], F32, tag="sc")
                for s0 in range(0, w, SUB):
                    sw = min(SUB, w - s0)
                    has_mask = (c0 + s0 + sw) > mask_sub0 * SUB
                    nc.tensor.matmul(
                        sc_ps[:, s0 : s0 + sw], lhsT=vrep_sb[:],
                        rhs=th[:, s0 : s0 + sw],
                        start=True, stop=not has_mask,
                    )
                    if has_mask:
                        nc.tensor.matmul(
                            sc_ps[:, s0 : s0 + sw], lhsT=ones1_sb[:],
                            rhs=msk_sb[:, j * L + c0 + s0 : j * L + c0 + s0 + sw],
                            start=False, stop=True,
                        )
                sh = smallp.tile([128, 1], F32, tag=f"sh{gi}")
                nc.scalar.activation(
                    p_sb[:, c0:c1], sc_ps[:, 0:w],
                    mybir.ActivationFunctionType.Exp, accum_out=sh[:],
                )
                s_parts.append(sh)

            # softmax denominator for this slot
            if len(s_parts) == 2:
                nc.gpsimd.tensor_add(s_all[:, j : j + 1], s_parts[0][:], s_parts[1][:])
            else:
                nc.gpsimd.tensor_copy(s_all[:, j : j + 1], s_parts[0][:])

            # stage B: one fused 2x multiply-accumulate per dc on the DVE;
            # the running total ripples into the zero-padded tail of scr
            for dc in range(DC):
                emit_mul_acc(
                    nc,
                    scr[:, dc, 0 : n_l + PADC],
                    et[:, dc, 0 : n_l + PADC],
                    p_sb[:, 0 : n_l + PADC],
                )
            # pull the settled totals out of the pad region, scaled by 1/S_j
            rj = smallp.tile([128, 1], F32, tag="rj")
            nc.vector.reciprocal(rj[:], s_all[:, j : j + 1])
            nc.vector.tensor_scalar_mul(
                res[:, j * DC : (j + 1) * DC],
                scr[:, :, n_l + PADC - 4],
                rj[:],
            )

        # transpose + write out (normalization already fused per slot)
        t_t = ps_parts.tile([A, GCOL], F32, tag="parts")
        t_ps = t_t[0 : BL * DC, 0:128]
        nc.tensor.transpose(t_ps, res[:], ident_sb[:])
        out_sb = resp.tile([BL * DC, 128], F32)
        nc.vector.tensor_copy(out_sb[:], t_ps)
        nc.sync.dma_start(out.ap().rearrange("b (dc x) -> (b dc) x", x=128), out_sb[:])

    nc.compile()
    return nc


_NC_CACHE = {}


def _get_nc(template):
    key = tuple((int(c), int(m)) for c, m in template)
    if key not in _NC_CACHE:
        _NC_CACHE[key] = _build_bass(key)
    return _NC_CACHE[key]


def _plan(lens):
    """Balance batches across cores by valid-chunk count.

    Returns (assign, template): assign[c][j] = original batch index handled
    by core c, slot j; template[j] = (chunks, first_mask_sub) compiled for
    slot j (chunks = max need over cores; first_mask_sub = first 512-col
    subchunk index containing any invalid position for any core's batch).
    """
    lens = np.maximum(np.asarray(lens), 1)
    chunks = np.minimum(np.ceil(lens / CHUNK).astype(int), NCH)
    order = np.argsort(-chunks, kind="stable")  # descending need
    # rank r (0=biggest group) -> slot position: put the second-smallest
    # group first so the pipeline primes fast, keep the smallest last so the
    # drain tail is short, biggest groups in the middle.
    ranks = list(range(BL))
    slot_ranks = [ranks[-2]] + ranks[: BL - 2] + [ranks[-1]]
    assign = [
        [int(order[r * N_CORES + c]) for r in slot_ranks] for c in range(N_CORES)
    ]
    template = []
    for si, r in enumerate(slot_ranks):
        group = [int(order[r * N_CORES + c]) for c in range(N_CORES)]
        cmax = int(chunks[order[r * N_CORES]])
        min_len = int(min(lens[b] for b in group))
        template.append((cmax, min_len // SUB))
    return assign, tuple(template)


def prepare_in_maps(enc_outputs, lens, hidden_states, W_enc, b_attn, W_hidden, v):
    """Host-side sharding + layout transforms. Returns (in_maps, assign)."""
    enc_outputs = np.asarray(enc_outputs, dtype=np.float32)
    lens = np.asarray(lens, dtype=np.int32)
    hidden_states = np.asarray(hidden_states, dtype=np.float32)
    W_enc = np.asarray(W_enc, dtype=np.float32)
    b_attn = np.asarray(b_attn, dtype=np.float32)
    W_hidden = np.asarray(W_hidden, dtype=np.float32)
    v = np.asarray(v, dtype=np.float32)

    assign, template = _plan(lens)

    # (L, B, D) -> (B, D, L), contiguous, fp16 (halves the HBM traffic; the
    # softmax weights and stage-B accumulation stay bf16/fp32)
    encT = np.ascontiguousarray(enc_outputs.transpose(1, 2, 0).astype(np.float16))
    w_enc_r = W_enc.astype(np.float16)
    vrep = np.ascontiguousarray(np.repeat(v.astype(np.float16)[:, None], 128, axis=1))
    ones1 = np.ones((1, 128), dtype=ml_dtypes.float8_e5m2)
    ident = np.eye(128, dtype=np.float32)
    b_attn_c = np.ascontiguousarray(b_attn[:, None])

    # length mask rows: 0 where l < lens[b], -30000 where l >= lens[b]
    li = np.arange(L, dtype=np.int32)[None, :]
    mask_full = np.where(li < lens[:, None], 0.0, -30000.0).astype(
        ml_dtypes.float8_e5m2
    )  # (B, L)

    hiddenT = hidden_states.T  # (H, B)

    in_maps = []
    for c in range(N_CORES):
        bs = assign[c]
        in_maps.append(
            {
                "encT": np.ascontiguousarray(encT[bs]).reshape(BL * D, L),
                "msk": np.ascontiguousarray(mask_full[bs]).reshape(1, BL * L),
                "hidT": np.ascontiguousarray(hiddenT[:, bs]),
                "w_enc": w_enc_r,
                "w_hid": W_hidden,
                "b_attn": b_attn_c,
                "vrep": vrep,
                "ones1": ones1,
                "ident": ident,
            }
        )
    return in_maps, assign, template


def _run(inputs_np, trace=False):
    in_maps, assign, template = prepare_in_maps(**inputs_np)
    nc = _get_nc(template)
    res = run_bass_kernel_spmd(
        nc, in_maps, core_ids=list(range(N_CORES)), trace=trace
    )
    out = np.empty((B, D), dtype=np.float32)
    for c in range(N_CORES):
        rows = res.results[c]["out"]
        for j in range(BL):
            out[assign[c][j]] = rows[j]
    return out, res


def kernel(enc_outputs, lens, hidden_states, W_enc, b_attn, W_hidden, v, **kwargs):
    out, _ = _run(
        dict(
            enc_outputs=enc_outputs, lens=lens, hidden_states=hidden_states,
            W_enc=W_enc, b_attn=b_attn, W_hidden=W_hidden, v=v,
        )
    )
    return out


def kernel_traced(enc_outputs, lens, hidden_states, W_enc, b_attn, W_hidden, v):
    """Like kernel() but returns (output, BassKernelResults with trace)."""
    return _run(
        dict(
            enc_outputs=enc_outputs, lens=lens, hidden_states=hidden_states,
            W_enc=W_enc, b_attn=b_attn, W_hidden=W_hidden, v=v,
        ),
        trace=True,
    )
